# revision 1
# baseline (speedup 1.0000x reference)
"""Block-sparse attention Trainium2 kernel.

Problem: nn_BlockSparseAttention (B=4, N=8256=64x129 tokens, D=1024,
H=8 heads, DK=DV=64, BLK=129). Full computation:
  q,k,v = x@Wq, x@Wk, x@Wv (per-head reshape)
  block-local softmax attention within each 129-token block
  global attention: slot-0 token of each block attends over all blocks'
  slot-0 tokens; its output is *added* to the local output at slot 0
  y = out @ Wo + bo

Sharding: 64 blocks split 8 ways (8 contiguous blocks per core, all 4
batches). Global-token K/V (64 tokens/batch) are computed redundantly on
every core from an xg input (the slot-0 rows of x), so no collectives are
needed. Each core returns its [4, 1032, 1024] slice of y.

On-device pipeline (all matmuls bf16 inputs, fp32 PSUM accumulation):
  - x is DMA-loaded with fp32->bf16 cast (SWDGE), transposed on the PE
    (via identity matmul) into xT [D, tokens] layout.
  - qT/kT = W^T @ xT stay feature-on-partition; v = x@Wv token-on-partition.
  - scores are computed transposed, sT[j, i] = k_j . q_i, so the
    attention-weights matmul (PV) needs no transposes; softmax denominators
    come from a ones-vector matmul; exp runs on the scalar engine reading
    PSUM directly (scale=1/sqrt(DK) folded in). Scores here are O(1) so the
    max-subtraction is skipped (exp is safe in fp32).
  - normalization multiplies the PV output by a broadcast reciprocal
    (broadcast across partitions via a tiny 2-row selector matmul).
  - y = outT^T @ Wo + bo, bias added during the PSUM->SBUF copy.
"""

import numpy as np

H, BLK, DK, DV = 8, 129, 64, 64
B, N, D = 4, 8256, 1024
INNER = H * DK            # 512
NB = N // BLK             # 64 blocks
NCORES = 8
NBC = NB // NCORES        # 8 blocks per core
T = NBC * BLK             # 1032 tokens per core per batch

_NC_CACHE = {}


def _build_nc(batches=B, do_attn=True, do_global=True, do_last=True, do_pv=True, parts=31):
    import concourse.bacc as bacc
    import concourse.tile as tile
    from concourse import mybir
    import concourse.bass as bass
    from concourse.masks import make_identity

    f32 = mybir.dt.float32
    bf16 = mybir.dt.bfloat16

    nc = bacc.Bacc("TRN2", target_bir_lowering=False, debug=False,
                   num_devices=NCORES)

    xc = nc.dram_tensor("xc", [B, T, D], f32, kind="ExternalInput").ap()
    xg = nc.dram_tensor("xg", [B, NB, D], f32, kind="ExternalInput").ap()
    wq = nc.dram_tensor("wq", [D, INNER], f32, kind="ExternalInput").ap()
    wk = nc.dram_tensor("wk", [D, INNER], f32, kind="ExternalInput").ap()
    wv = nc.dram_tensor("wv", [D, INNER], f32, kind="ExternalInput").ap()
    wo = nc.dram_tensor("wo", [INNER, D], f32, kind="ExternalInput").ap()
    bo = nc.dram_tensor("bo", [1, D], f32, kind="ExternalInput").ap()
    y = nc.dram_tensor("y", [B, T, D], f32, kind="ExternalOutput").ap()

    DC = D // 128             # 8 contraction chunks over D
    FC = INNER // 128         # 4 chunks over the 512 inner dim
    # token slices for the projection matmuls (psum free dim <= 512)
    TSL = [(0, 512), (512, 512), (1024, T - 1024)]
    # token chunks for x load/transpose and the output projection
    TCH = [(i * 128, 128) for i in range(T // 128)] + [(T - T % 128, T % 128)]

    with tile.TileContext(nc) as tc:
        with (
            tc.tile_pool(name="const", bufs=1) as const,
            tc.tile_pool(name="batch", bufs=2) as bp,
            tc.tile_pool(name="stream", bufs=3) as sp,
            tc.tile_pool(name="att", bufs=3) as ap_,
            tc.tile_pool(name="ppsum", bufs=3, space="PSUM") as pp,
            tc.tile_pool(name="spsum", bufs=2, space="PSUM") as stp,
            tc.tile_pool(name="smpsum", bufs=3, space="PSUM") as smp,
        ):
            # ---- constants ----
            ident = const.tile([128, 128], bf16)
            make_identity(nc, ident)
            ones_col = const.tile([128, 1], bf16)
            nc.vector.memset(ones_col, 1.0)
            ones_row = const.tile([1, 128], bf16)
            nc.vector.memset(ones_row, 1.0)
            # E2: partition-broadcast selector. E2[0, 0:64]=1, E2[1, 64:128]=1
            import ml_dtypes
            e2_np = np.zeros((2, 128), dtype=ml_dtypes.bfloat16)
            e2_np[0, 0:64] = 1.0
            e2_np[1, 64:128] = 1.0
            e2_dram = nc.inline_tensor(e2_np, name="e2const")
            e2 = const.tile([2, 128], bf16)
            nc.sync.dma_start(out=e2, in_=e2_dram.ap())

            wq_sb = const.tile([128, DC, INNER], bf16)
            wk_sb = const.tile([128, DC, INNER], bf16)
            wv_sb = const.tile([128, DC, INNER], bf16)
            wo_sb = const.tile([128, FC, D], bf16)
            nc.gpsimd.dma_start(
                out=wv_sb, in_=wv.rearrange("(c p) f -> p c f", p=128))
            # Wq/Wk loaded with heads interleaved: stored col m*128+64*a+d
            # holds original col 256*a+64*m+d, so head h lives at
            # (chunk h%4, partition base 64*(h//4)).
            for w_sb, w in ((wq_sb, wq), (wk_sb, wk)):
                w_v = w.rearrange("(c p) (a m d) -> p c a m d",
                                  p=128, a=2, d=64)
                for a2 in range(2):
                    for cc in range(DC):
                        nc.gpsimd.dma_start(
                            out=w_sb[:, cc, :].rearrange(
                                "p (m x) -> p m x",
                                x=128)[:, :, 64 * a2:64 * a2 + 64],
                            in_=w_v[:, cc, a2, :, :])
            nc.gpsimd.dma_start(
                out=wo_sb, in_=wo.rearrange("(c p) f -> p c f", p=128))
            bo_bc = const.tile([128, D], f32)
            nc.gpsimd.dma_start(
                out=bo_bc,
                in_=bass.AP(tensor=bo.tensor, offset=bo.offset,
                            ap=[[0, 128], [1, D]]))

            for b in range(batches):
                # ---- load + transpose x for this batch ----
                xT = bp.tile([128, DC, T], bf16, tag="xT")
                for t0, tsz in TCH:
                    xch = sp.tile([128, D], bf16, tag="xch")
                    nc.gpsimd.dma_start(out=xch[:tsz, :],
                                        in_=xc[b, t0:t0 + tsz, :])
                    for dc in range(DC):
                        pt = pp.tile([128, 512], bf16, tag="pp")
                        nc.tensor.transpose(
                            pt[:, :tsz],
                            xch[:tsz, dc * 128:(dc + 1) * 128],
                            ident[:tsz, :tsz])
                        nc.scalar.copy(out=xT[:, dc, t0:t0 + tsz],
                                       in_=pt[:, :tsz])

                # ---- global tokens: xgT, kgT, vg ----
                xgs = bp.tile([64, D], bf16, tag="xgs")
                nc.gpsimd.dma_start(out=xgs, in_=xg[b])
                xgT = bp.tile([128, DC, NB], bf16, tag="xgT")
                for dc in range(DC):
                    pt = pp.tile([128, 512], bf16, tag="pp")
                    nc.tensor.transpose(
                        pt[:, :NB], xgs[:, dc * 128:(dc + 1) * 128],
                        ident[:NB, :NB])
                    nc.scalar.copy(out=xgT[:, dc, :], in_=pt[:, :NB])
                kgT = bp.tile([128, FC, NB], bf16, tag="kgT")
                for mc in range(FC):
                    pt = pp.tile([128, 512], f32, tag="pp")
                    for dc in range(DC):
                        nc.tensor.matmul(
                            pt[:, :NB],
                            wk_sb[:, dc, mc * 128:(mc + 1) * 128],
                            xgT[:, dc, :],
                            start=(dc == 0), stop=(dc == DC - 1))
                    nc.vector.tensor_copy(out=kgT[:, mc, :], in_=pt[:, :NB])
                vg = bp.tile([64, INNER], bf16, tag="vg")
                pt = pp.tile([128, 512], f32, tag="pp")
                for dc in range(DC):
                    nc.tensor.matmul(pt[:64, :], xgT[:, dc, 0:64],
                                     wv_sb[:, dc, :],
                                     start=(dc == 0), stop=(dc == DC - 1))
                nc.vector.tensor_copy(out=vg, in_=pt[:64, :])

                # ---- q/k projections (transposed layout) ----
                qT = bp.tile([128, FC, T], bf16, tag="qT")
                kT = bp.tile([128, FC, T], bf16, tag="kT")
                for dst, w_sb, eng in ((qT, wq_sb, "act"), (kT, wk_sb, "dve")):
                    for mc in range(FC):
                        for t0, tsz in TSL:
                            pt = pp.tile([128, 512], f32, tag="pp")
                            for dc in range(DC):
                                nc.tensor.matmul(
                                    pt[:, :tsz],
                                    w_sb[:, dc, mc * 128:(mc + 1) * 128],
                                    xT[:, dc, t0:t0 + tsz],
                                    start=(dc == 0), stop=(dc == DC - 1))
                            if eng == "act":
                                nc.scalar.copy(
                                    out=dst[:, mc, t0:t0 + tsz],
                                    in_=pt[:, :tsz])
                            else:
                                nc.vector.tensor_copy(
                                    out=dst[:, mc, t0:t0 + tsz],
                                    in_=pt[:, :tsz])

                # ---- v projection (token-on-partition, per block) ----
                v = bp.tile([128, NBC, INNER], bf16, tag="v")
                for n in range(NBC):
                    pt = pp.tile([128, 512], f32, tag="pp")
                    for dc in range(DC):
                        nc.tensor.matmul(
                            pt, xT[:, dc, n * BLK:n * BLK + 128],
                            wv_sb[:, dc, :],
                            start=(dc == 0), stop=(dc == DC - 1))
                    nc.vector.tensor_copy(out=v[:, n, :], in_=pt)
                # last token of each block, batched: tokens 129n+128
                vl8 = bp.tile([NBC, INNER], bf16, tag="vl8")
                pt = pp.tile([128, 512], f32, tag="pp")
                for dc in range(DC):
                    nc.tensor.matmul(pt[:NBC, :], xT[:, dc, 128::BLK],
                                     wv_sb[:, dc, :],
                                     start=(dc == 0), stop=(dc == DC - 1))
                nc.vector.tensor_copy(out=vl8, in_=pt[:NBC, :])
                vl_all = bp.tile([1, NBC, INNER], bf16, tag="vlall")
                nc.sync.dma_start(out=vl_all, in_=vl8)

                outT = bp.tile([128, FC, T], bf16, tag="outT")
                if (not do_attn) or not (parts & 16):
                    nc.vector.memset(outT, 0.0)

                # ---- global attention for this core's 8 blocks ----
                if do_global:
                    eg = bp.tile([64, H, NBC], bf16, tag="eg")
                    lg = smp.tile([1, H * NBC], f32, tag="sm")
                    for h in range(H):
                        p0 = 64 * (h // 4)
                        hc = h % 4
                        sg = smp.tile([64, NBC], f32, tag="sm")
                        nc.tensor.matmul(sg, kgT[p0:p0 + 64, hc, :],
                                         qT[p0:p0 + 64, hc, 0::BLK],
                                         start=True, stop=True)
                        nc.scalar.activation(
                            out=eg[:, h, :], in_=sg,
                            func=mybir.ActivationFunctionType.Exp, scale=0.125)
                        nc.tensor.matmul(lg[:, h * NBC:(h + 1) * NBC],
                                         ones_col[0:64, :], eg[:, h, :],
                                         start=True, stop=True)
                    rlg = bp.tile([1, H * NBC], bf16, tag="rlg")
                    with nc.allow_low_precision("1/l to bf16"):
                        nc.vector.reciprocal(out=rlg, in_=lg)
                    ogn = bp.tile([128, FC, NBC], bf16, tag="ogn")
                    for hp in range(4):
                        ogg = smp.tile([128, NBC], f32, tag="sm")
                        for hh in range(2):
                            h = 2 * hp + hh
                            nc.tensor.matmul(
                                ogg[64 * hh:64 * hh + 64, :],
                                vg[:, h * DV:(h + 1) * DV], eg[:, h, :],
                                start=True, stop=True)
                        rlbg = smp.tile([128, NBC], f32, tag="sm")
                        for hh in range(2):
                            o0 = hp * 2 * NBC + hh * NBC
                            nc.tensor.matmul(
                                rlbg[64 * hh:64 * hh + 64, :],
                                ones_row[0:1, 0:64],
                                rlg[0:1, o0:o0 + NBC],
                                start=True, stop=True)
                        rlbg_sb = bp.tile([128, NBC], bf16, tag="rlbg_sb")
                        nc.scalar.copy(out=rlbg_sb, in_=rlbg)
                        nc.vector.tensor_mul(out=ogn[:, hp, :], in0=ogg,
                                             in1=rlbg_sb)


                # ---- block-local attention ----
                for n in range(NBC if do_attn else 0):
                    c0 = n * BLK
                    eT = ap_.tile([128, H, BLK], bf16, tag="eT")
                    eTl = ap_.tile([1, H, BLK], bf16, tag="eTl")
                    rl = ap_.tile([1, H * BLK], bf16, tag="rl")
                    if not (parts & 65):
                        nc.vector.memset(eT, 0.001)
                        nc.vector.memset(eTl, 0.001)
                    for hp in range(4):
                        st = stp.tile([128, 2 * BLK], f32, tag="st")
                        stl = smp.tile([1, 2 * BLK], f32, tag="sm")
                        if (parts & 64) and not (parts & 33):
                            nc.vector.memset(st, 0.5)
                            nc.vector.memset(stl, 0.5)
                        for hh in range(2 if (parts & 33) else 0):
                            h = 2 * hp + hh
                            p0 = 64 * (h // 4)
                            hc = h % 4
                            lq = qT[p0:p0 + 64, hc, c0:c0 + BLK]
                            nc.tensor.matmul(
                                st[:, hh * BLK:(hh + 1) * BLK],
                                kT[p0:p0 + 64, hc, c0:c0 + 128], lq,
                                start=True, stop=True)
                            if do_last:
                                nc.tensor.matmul(
                                    stl[:, hh * BLK:(hh + 1) * BLK],
                                    kT[p0:p0 + 64, hc, c0 + 128:c0 + BLK], lq,
                                    start=True, stop=True)
                        ex = mybir.ActivationFunctionType.Exp
                        if parts & 65:
                            nc.scalar.activation(
                                out=eT[:, 2 * hp:2 * hp + 2, :], in_=st,
                                func=ex, scale=0.125)
                        if do_last and (parts & 65):
                            nc.scalar.activation(
                                out=eTl[:, 2 * hp:2 * hp + 2, :], in_=stl,
                                func=ex, scale=0.125)
                        if (parts & 32) and not (parts & 65):
                            nc.vector.memset(eT[:, 2 * hp:2 * hp + 2, :], 0.001)
                            nc.vector.memset(eTl[:, 2 * hp:2 * hp + 2, :], 0.001)
                        if (not do_last) and (parts & 65):
                            nc.vector.memset(eTl[:, 2 * hp:2 * hp + 2, :], 0.0)
                        if parts & 2:
                            lp = smp.tile([1, 2 * BLK], f32, tag="sm")
                            nc.tensor.matmul(lp, ones_col,
                                             eT[:, 2 * hp:2 * hp + 2, :],
                                             start=True, stop=not do_last)
                            if do_last:
                                nc.tensor.matmul(lp, ones_col[0:1, :],
                                                 eTl[:, 2 * hp:2 * hp + 2, :],
                                                 start=False, stop=True)
                            with nc.allow_low_precision(
                                    "1/l to bf16, matches prior cast-DMA"):
                                nc.vector.reciprocal(
                                    out=rl[:, hp * 2 * BLK:(hp + 1) * 2 * BLK],
                                    in_=lp)
                    # split rl [1, H*BLK] -> [2, 4, BLK] (pair-member on
                    # partition) with bf16 cast, via SWDGE reshape DMA
                    if not (parts & 2):
                        nc.vector.memset(rl, 1.0)
                    for hp in range(4):
                        og = smp.tile([128, BLK], f32, tag="sm")
                        if not (parts & 8):
                            nc.vector.memset(og, 0.0)
                        for hh in range(2 if (parts & 8) else 0):
                            h = 2 * hp + hh
                            nc.tensor.matmul(
                                og[64 * hh:64 * hh + 64, :],
                                v[:, n, h * DV:(h + 1) * DV],
                                eT[:, h, :], start=True,
                                stop=not (do_last and do_pv))
                            if do_last and do_pv:
                                nc.tensor.matmul(
                                    og[64 * hh:64 * hh + 64, :],
                                    vl_all[0:1, n, h * DV:(h + 1) * DV],
                                    eTl[:, h, :], start=False, stop=True)
                        rlb_sb = ap_.tile([128, BLK], bf16, tag="rlb_sb")
                        if parts & 4:
                            rlb = smp.tile([128, BLK], f32, tag="sm")
                            for hh in range(2):
                                o0 = hp * 2 * BLK + hh * BLK
                                nc.tensor.matmul(
                                    rlb[64 * hh:64 * hh + 64, :],
                                    ones_row[0:1, 0:64],
                                    rl[0:1, o0:o0 + BLK],
                                    start=True, stop=True)
                            nc.scalar.copy(out=rlb_sb, in_=rlb)
                        else:
                            nc.vector.memset(rlb_sb, 1.0)
                        if parts & 16:
                            nc.vector.tensor_mul(
                                out=outT[:, hp, c0:c0 + BLK], in0=og,
                                in1=rlb_sb)
                            if do_global:
                                nc.vector.tensor_add(
                                    out=outT[:, hp, c0:c0 + 1],
                                    in0=outT[:, hp, c0:c0 + 1],
                                    in1=ogn[:, hp, n:n + 1])

                # ---- output projection + bias ----
                for t0, tsz in TCH:
                    ysb = sp.tile([128, D], f32, tag="ysb")
                    for half in range(2):
                        f0 = half * 512
                        pt = pp.tile([128, 512], f32, tag="pp")
                        for fc in range(FC):
                            nc.tensor.matmul(
                                pt[:tsz, :],
                                outT[:, fc, t0:t0 + tsz],
                                wo_sb[:, fc, f0:f0 + 512],
                                start=(fc == 0), stop=(fc == FC - 1))
                        nc.vector.tensor_add(
                            out=ysb[:tsz, f0:f0 + 512], in0=pt[:tsz, :],
                            in1=bo_bc[:tsz, f0:f0 + 512])
                    nc.sync.dma_start(out=y[b, t0:t0 + tsz, :],
                                      in_=ysb[:tsz, :])

    nc.compile()
    return nc


def _get_nc():
    if "nc" not in _NC_CACHE:
        _NC_CACHE["nc"] = _build_nc()
    return _NC_CACHE["nc"]


def kernel(x, Wq, Wk, Wv, Wo, bo):
    from concourse.bass_utils import run_bass_kernel_spmd

    x = np.asarray(x, dtype=np.float32)
    nc = _get_nc()
    xg = np.ascontiguousarray(x[:, ::BLK, :])
    bo2 = np.asarray(bo, dtype=np.float32).reshape(1, D)
    in_maps = []
    for c in range(NCORES):
        in_maps.append({
            "xc": np.ascontiguousarray(x[:, c * T:(c + 1) * T, :]),
            "xg": xg,
            "wq": np.asarray(Wq, np.float32),
            "wk": np.asarray(Wk, np.float32),
            "wv": np.asarray(Wv, np.float32),
            "wo": np.asarray(Wo, np.float32),
            "bo": bo2,
        })
    res = run_bass_kernel_spmd(nc, in_maps, core_ids=list(range(NCORES)))
    return np.concatenate([res.results[c]["y"] for c in range(NCORES)],
                          axis=1)



# revision 5
# speedup vs baseline: 58.6919x; 58.6919x over previous
"""Block-sparse attention Trainium2 kernel.

Problem: nn_BlockSparseAttention (B=4, N=8256=64x129 tokens, D=1024,
H=8 heads, DK=DV=64, BLK=129). Full computation:
  q,k,v = x@Wq, x@Wk, x@Wv (per-head reshape)
  block-local softmax attention within each 129-token block
  global attention: slot-0 token of each block attends over all blocks'
  slot-0 tokens; its output is *added* to the local output at slot 0
  y = out @ Wo + bo

Sharding: 64 blocks split 8 ways (8 contiguous blocks per core, all 4
batches). Global-token K/V (64 tokens/batch) are computed redundantly on
every core from the slot-0 rows of x, so no collectives are needed.
Each core returns its [4, 1032, 1024] slice of y.

Host-side prep (pure layout/dtype transforms, no model math): the x
slice is cast to bf16 and pre-transposed to feature-on-partition layout
[128, D/128, T]; weights are cast to bf16 and pre-arranged into the
SBUF layouts the matmuls want (Wq/Wk head-interleaved); everything is
packed into ONE flat bf16 input buffer per core (fewer buffer handles
per dispatch = less per-call marshalling).

On-device pipeline (all matmuls bf16 inputs, fp32 PSUM accumulation):
  - qT/kT = W^T @ xT stay feature-on-partition; v = x@Wv
    token-on-partition.
  - scores are computed transposed, sT[j, i] = k_j . q_i, so the
    attention-weights matmul (PV) needs no transposes; softmax
    denominators come from a ones-vector matmul; exp runs on the scalar
    engine reading PSUM directly (scale=1/sqrt(DK) folded in). Scores
    here are O(1) so the max-subtraction is skipped (exp safe in fp32).
  - normalization multiplies the PV output by a broadcast reciprocal
    (broadcast across partitions via a tiny selector matmul).
  - y = outT^T @ Wo + bo, bias added during the PSUM->SBUF copy;
    y is stored bf16 and upcast to fp32 on the host.
"""

import numpy as np

H, BLK, DK, DV = 8, 129, 64, 64
B, N, D = 4, 8256, 1024
INNER = H * DK            # 512
NB = N // BLK             # 64 blocks
NCORES = 8
NBC = NB // NCORES        # 8 blocks per core
T = NBC * BLK             # 1032 tokens per core per batch
DC = D // 128             # 8 contraction chunks over D
FC = INNER // 128         # 4 chunks over the 512 inner dim

# pack buffer element offsets (bf16 elements)
SZ_XCT = B * 128 * DC * T
SZ_XGT = B * 128 * DC * NB
SZ_W = 128 * DC * INNER   # == 128 * FC * D for wo
OFF_XCT = 0
OFF_XGT = OFF_XCT + SZ_XCT
OFF_WQ = OFF_XGT + SZ_XGT
OFF_WK = OFF_WQ + SZ_W
OFF_WV = OFF_WK + SZ_W
OFF_WO = OFF_WV + SZ_W
OFF_BO = OFF_WO + SZ_W
PACK_TOTAL = OFF_BO + D

_NC_CACHE = {}


def _build_nc():
    import concourse.bacc as bacc
    import concourse.tile as tile
    from concourse import mybir
    import concourse.bass as bass

    f32 = mybir.dt.float32
    bf16 = mybir.dt.bfloat16

    nc = bacc.Bacc("TRN2", target_bir_lowering=False, debug=False,
                   num_devices=NCORES)

    pack = nc.dram_tensor("pack", [PACK_TOTAL], bf16,
                          kind="ExternalInput").ap()
    y = nc.dram_tensor("y", [B, T, D], bf16, kind="ExternalOutput").ap()

    def sub(off, dims):
        """AP view into the flat pack: dims = [(stride, size), ...]."""
        return bass.AP(tensor=pack.tensor, offset=off,
                       ap=[[s, n] for s, n in dims])

    # token slices for the projection matmuls (psum free dim <= 512)
    TSL = [(0, 512), (512, 512), (1024, T - 1024)]
    # token chunks for the output projection
    TCH = [(i * 128, 128) for i in range(T // 128)] + [(T - T % 128, T % 128)]

    with tile.TileContext(nc) as tc:
        with (
            tc.tile_pool(name="const", bufs=1) as const,
            tc.tile_pool(name="batch", bufs=2) as bp,
            tc.tile_pool(name="stream", bufs=3) as sp,
            tc.tile_pool(name="att", bufs=3) as ap_,
            tc.tile_pool(name="ppsum", bufs=3, space="PSUM") as pp,
            tc.tile_pool(name="spsum", bufs=2, space="PSUM") as stp,
            tc.tile_pool(name="smpsum", bufs=3, space="PSUM") as smp,
        ):
            # ---- constants ----
            ones_col = const.tile([128, 1], bf16)
            nc.vector.memset(ones_col, 1.0)
            ones_row = const.tile([1, 128], bf16)
            nc.vector.memset(ones_row, 1.0)

            wq_sb = const.tile([128, DC, INNER], bf16)
            wk_sb = const.tile([128, DC, INNER], bf16)
            wv_sb = const.tile([128, DC, INNER], bf16)
            wo_sb = const.tile([128, FC, D], bf16)
            for w_sb, off in ((wq_sb, OFF_WQ), (wk_sb, OFF_WK)):
                nc.sync.dma_start(
                    out=w_sb,
                    in_=sub(off, [(DC * INNER, 128), (INNER, DC),
                                  (1, INNER)]))
            nc.scalar.dma_start(
                out=wv_sb,
                in_=sub(OFF_WV, [(DC * INNER, 128), (INNER, DC), (1, INNER)]))
            nc.scalar.dma_start(
                out=wo_sb,
                in_=sub(OFF_WO, [(FC * D, 128), (D, FC), (1, D)]))
            bo_bc = const.tile([128, D], bf16)
            nc.sync.dma_start(out=bo_bc, in_=sub(OFF_BO, [(0, 128), (1, D)]))

            for b in range(B):
                # ---- load pre-transposed x / global tokens ----
                xT = bp.tile([128, DC, T], bf16, tag="xT")
                nc.sync.dma_start(
                    out=xT,
                    in_=sub(OFF_XCT + b * 128 * DC * T,
                            [(DC * T, 128), (T, DC), (1, T)]))
                xgT = bp.tile([128, DC, NB], bf16, tag="xgT")
                nc.scalar.dma_start(
                    out=xgT,
                    in_=sub(OFF_XGT + b * 128 * DC * NB,
                            [(DC * NB, 128), (NB, DC), (1, NB)]))

                # ---- global tokens: kgT, vg ----
                kgT = bp.tile([128, FC, NB], bf16, tag="kgT")
                for mc in range(FC):
                    pt = pp.tile([128, 512], f32, tag="pp")
                    for dc in range(DC):
                        nc.tensor.matmul(
                            pt[:, :NB],
                            wk_sb[:, dc, mc * 128:(mc + 1) * 128],
                            xgT[:, dc, :],
                            start=(dc == 0), stop=(dc == DC - 1))
                    nc.vector.tensor_copy(out=kgT[:, mc, :], in_=pt[:, :NB])
                vg = bp.tile([64, INNER], bf16, tag="vg")
                pt = pp.tile([128, 512], f32, tag="pp")
                for dc in range(DC):
                    nc.tensor.matmul(pt[:64, :], xgT[:, dc, 0:64],
                                     wv_sb[:, dc, :],
                                     start=(dc == 0), stop=(dc == DC - 1))
                nc.vector.tensor_copy(out=vg, in_=pt[:64, :])

                # ---- q/k projections (transposed layout) ----
                qT = bp.tile([128, FC, T], bf16, tag="qT")
                kT = bp.tile([128, FC, T], bf16, tag="kT")
                for dst, w_sb, eng in ((qT, wq_sb, "act"), (kT, wk_sb, "dve")):
                    for mc in range(FC):
                        for t0, tsz in TSL:
                            pt = pp.tile([128, 512], f32, tag="pp")
                            for dc in range(DC):
                                nc.tensor.matmul(
                                    pt[:, :tsz],
                                    w_sb[:, dc, mc * 128:(mc + 1) * 128],
                                    xT[:, dc, t0:t0 + tsz],
                                    start=(dc == 0), stop=(dc == DC - 1))
                            if eng == "act":
                                nc.scalar.copy(
                                    out=dst[:, mc, t0:t0 + tsz],
                                    in_=pt[:, :tsz])
                            else:
                                nc.vector.tensor_copy(
                                    out=dst[:, mc, t0:t0 + tsz],
                                    in_=pt[:, :tsz])

                # ---- v projection (token-on-partition, per block) ----
                v = bp.tile([128, NBC, INNER], bf16, tag="v")
                for n in range(NBC):
                    pt = pp.tile([128, 512], f32, tag="pp")
                    for dc in range(DC):
                        nc.tensor.matmul(
                            pt, xT[:, dc, n * BLK:n * BLK + 128],
                            wv_sb[:, dc, :],
                            start=(dc == 0), stop=(dc == DC - 1))
                    nc.vector.tensor_copy(out=v[:, n, :], in_=pt)
                # last token of each block, batched: tokens 129n+128
                vl8 = bp.tile([NBC, INNER], bf16, tag="vl8")
                pt = pp.tile([128, 512], f32, tag="pp")
                for dc in range(DC):
                    nc.tensor.matmul(pt[:NBC, :], xT[:, dc, 128::BLK],
                                     wv_sb[:, dc, :],
                                     start=(dc == 0), stop=(dc == DC - 1))
                nc.vector.tensor_copy(out=vl8, in_=pt[:NBC, :])
                vl_all = bp.tile([1, NBC, INNER], bf16, tag="vlall")
                nc.sync.dma_start(out=vl_all, in_=vl8)

                outT = bp.tile([128, FC, T], bf16, tag="outT")

                # ---- global attention for this core's 8 blocks ----
                eg = bp.tile([64, H, NBC], bf16, tag="eg")
                lg = smp.tile([1, H * NBC], f32, tag="sm")
                for h in range(H):
                    p0 = 64 * (h // 4)
                    hc = h % 4
                    sg = smp.tile([64, NBC], f32, tag="sm")
                    nc.tensor.matmul(sg, kgT[p0:p0 + 64, hc, :],
                                     qT[p0:p0 + 64, hc, 0::BLK],
                                     start=True, stop=True)
                    nc.scalar.activation(
                        out=eg[:, h, :], in_=sg,
                        func=mybir.ActivationFunctionType.Exp, scale=0.125)
                    nc.tensor.matmul(lg[:, h * NBC:(h + 1) * NBC],
                                     ones_col[0:64, :], eg[:, h, :],
                                     start=True, stop=True)
                rlg = bp.tile([1, H * NBC], bf16, tag="rlg")
                with nc.allow_low_precision("1/l to bf16"):
                    nc.vector.reciprocal(out=rlg, in_=lg)
                ogn = bp.tile([128, FC, NBC], bf16, tag="ogn")
                for hp in range(4):
                    ogg = smp.tile([128, NBC], f32, tag="sm")
                    for hh in range(2):
                        h = 2 * hp + hh
                        nc.tensor.matmul(
                            ogg[64 * hh:64 * hh + 64, :],
                            vg[:, h * DV:(h + 1) * DV], eg[:, h, :],
                            start=True, stop=True)
                    rlbg = smp.tile([128, NBC], f32, tag="sm")
                    for hh in range(2):
                        o0 = hp * 2 * NBC + hh * NBC
                        nc.tensor.matmul(
                            rlbg[64 * hh:64 * hh + 64, :],
                            ones_row[0:1, 0:64],
                            rlg[0:1, o0:o0 + NBC],
                            start=True, stop=True)
                    rlbg_sb = bp.tile([128, NBC], bf16, tag="rlbg_sb")
                    nc.scalar.copy(out=rlbg_sb, in_=rlbg)
                    nc.vector.tensor_mul(out=ogn[:, hp, :], in0=ogg,
                                         in1=rlbg_sb)

                # ---- block-local attention ----
                for n in range(NBC):
                    c0 = n * BLK
                    eT = ap_.tile([128, H, BLK], bf16, tag="eT")
                    eTl = ap_.tile([1, H, BLK], bf16, tag="eTl")
                    rl = ap_.tile([1, H * BLK], bf16, tag="rl")
                    for hp in range(4):
                        st = stp.tile([128, 2 * BLK], f32, tag="st")
                        stl = smp.tile([1, 2 * BLK], f32, tag="sm")
                        for hh in range(2):
                            h = 2 * hp + hh
                            p0 = 64 * (h // 4)
                            hc = h % 4
                            lq = qT[p0:p0 + 64, hc, c0:c0 + BLK]
                            nc.tensor.matmul(
                                st[:, hh * BLK:(hh + 1) * BLK],
                                kT[p0:p0 + 64, hc, c0:c0 + 128], lq,
                                start=True, stop=True)
                            nc.tensor.matmul(
                                stl[:, hh * BLK:(hh + 1) * BLK],
                                kT[p0:p0 + 64, hc, c0 + 128:c0 + BLK], lq,
                                start=True, stop=True)
                        ex = mybir.ActivationFunctionType.Exp
                        nc.scalar.activation(
                            out=eT[:, 2 * hp:2 * hp + 2, :], in_=st,
                            func=ex, scale=0.125)
                        nc.scalar.activation(
                            out=eTl[:, 2 * hp:2 * hp + 2, :], in_=stl,
                            func=ex, scale=0.125)
                        lp = smp.tile([1, 2 * BLK], f32, tag="sm")
                        nc.tensor.matmul(lp, ones_col,
                                         eT[:, 2 * hp:2 * hp + 2, :],
                                         start=True, stop=False)
                        nc.tensor.matmul(lp, ones_col[0:1, :],
                                         eTl[:, 2 * hp:2 * hp + 2, :],
                                         start=False, stop=True)
                        with nc.allow_low_precision(
                                "1/l to bf16, matches prior cast-DMA"):
                            nc.vector.reciprocal(
                                out=rl[:, hp * 2 * BLK:(hp + 1) * 2 * BLK],
                                in_=lp)
                    for hp in range(4):
                        og = smp.tile([128, BLK], f32, tag="sm")
                        for hh in range(2):
                            h = 2 * hp + hh
                            nc.tensor.matmul(
                                og[64 * hh:64 * hh + 64, :],
                                v[:, n, h * DV:(h + 1) * DV],
                                eT[:, h, :], start=True, stop=False)
                            nc.tensor.matmul(
                                og[64 * hh:64 * hh + 64, :],
                                vl_all[0:1, n, h * DV:(h + 1) * DV],
                                eTl[:, h, :], start=False, stop=True)
                        rlb = smp.tile([128, BLK], f32, tag="sm")
                        for hh in range(2):
                            o0 = hp * 2 * BLK + hh * BLK
                            nc.tensor.matmul(
                                rlb[64 * hh:64 * hh + 64, :],
                                ones_row[0:1, 0:64],
                                rl[0:1, o0:o0 + BLK],
                                start=True, stop=True)
                        rlb_sb = ap_.tile([128, BLK], bf16, tag="rlb_sb")
                        nc.scalar.copy(out=rlb_sb, in_=rlb)
                        nc.vector.tensor_mul(
                            out=outT[:, hp, c0:c0 + BLK], in0=og,
                            in1=rlb_sb)
                        nc.vector.tensor_add(
                            out=outT[:, hp, c0:c0 + 1],
                            in0=outT[:, hp, c0:c0 + 1],
                            in1=ogn[:, hp, n:n + 1])

                # ---- output projection + bias ----
                for t0, tsz in TCH:
                    ysb = sp.tile([128, D], bf16, tag="ysb")
                    for half in range(2):
                        f0 = half * 512
                        pt = pp.tile([128, 512], f32, tag="pp")
                        for fc in range(FC):
                            nc.tensor.matmul(
                                pt[:tsz, :],
                                outT[:, fc, t0:t0 + tsz],
                                wo_sb[:, fc, f0:f0 + 512],
                                start=(fc == 0), stop=(fc == FC - 1))
                        with nc.allow_low_precision("y stored bf16"):
                            nc.vector.tensor_add(
                                out=ysb[:tsz, f0:f0 + 512], in0=pt[:tsz, :],
                                in1=bo_bc[:tsz, f0:f0 + 512])
                    nc.sync.dma_start(out=y[b, t0:t0 + tsz, :],
                                      in_=ysb[:tsz, :])

    nc.compile()
    return nc


def _get_nc():
    if "nc" not in _NC_CACHE:
        _NC_CACHE["nc"] = _build_nc()
    return _NC_CACHE["nc"]


def prepare_in_maps(x, Wq, Wk, Wv, Wo, bo):
    """Host-side layout/dtype prep: returns per-core {'pack': flat bf16}."""
    import ml_dtypes

    bf = ml_dtypes.bfloat16
    x = np.asarray(x, dtype=np.float32)
    Wq, Wk, Wv, Wo = (np.asarray(w, np.float32) for w in (Wq, Wk, Wv, Wo))
    bo = np.asarray(bo, np.float32).reshape(D)

    def qk_interleave(W):
        # w_sb[p, c, m*128 + 64*a + d] = W[c*128 + p, a*256 + m*64 + d]
        Wv5 = W.reshape(DC, 128, 2, 4, 64)
        return np.ascontiguousarray(
            Wv5.transpose(1, 0, 3, 2, 4)).reshape(128, DC * INNER).astype(bf)

    wq_i = qk_interleave(Wq)
    wk_i = qk_interleave(Wk)
    wv_s = np.ascontiguousarray(
        Wv.reshape(DC, 128, INNER).transpose(1, 0, 2)).reshape(
            128, DC * INNER).astype(bf)
    wo_s = np.ascontiguousarray(
        Wo.reshape(FC, 128, D).transpose(1, 0, 2)).reshape(
            128, FC * D).astype(bf)
    bo_b = bo.astype(bf)

    xg = x[:, ::BLK, :]                                   # [B, NB, D]
    xgt = np.ascontiguousarray(
        xg.reshape(B, NB, DC, 128).transpose(0, 3, 2, 1)).astype(bf)

    const_part = np.concatenate(
        [a.reshape(-1) for a in (wq_i, wk_i, wv_s, wo_s, bo_b)])

    in_maps = []
    for c in range(NCORES):
        xc = x[:, c * T:(c + 1) * T, :]                   # [B, T, D]
        xct = np.ascontiguousarray(
            xc.reshape(B, T, DC, 128).transpose(0, 3, 2, 1)).astype(bf)
        packv = np.concatenate(
            [xct.reshape(-1), xgt.reshape(-1), const_part])
        assert packv.shape[0] == PACK_TOTAL
        in_maps.append({"pack": packv})
    return in_maps


def kernel(x, Wq, Wk, Wv, Wo, bo):
    from concourse.bass_utils import run_bass_kernel_spmd

    nc = _get_nc()
    in_maps = prepare_in_maps(x, Wq, Wk, Wv, Wo, bo)
    res = run_bass_kernel_spmd(nc, in_maps, core_ids=list(range(NCORES)))
    return np.concatenate(
        [res.results[c]["y"] for c in range(NCORES)],
        axis=1).astype(np.float32)


# revision 16
# speedup vs baseline: 75.9504x; 1.2941x over previous
"""Block-sparse attention Trainium2 kernel.

Problem: nn_BlockSparseAttention (B=4, N=8256=64x129 tokens, D=1024,
H=8 heads, DK=DV=64, BLK=129). Full computation:
  q,k,v = x@Wq, x@Wk, x@Wv (per-head reshape)
  block-local softmax attention within each 129-token block
  global attention: slot-0 token of each block attends over all blocks'
  slot-0 tokens; its output is *added* to the local output at slot 0
  y = out @ Wo + bo

Sharding: 64 blocks split 8 ways (8 contiguous blocks per core, all 4
batches). Global-token K/V (64 tokens/batch) are computed redundantly on
every core from the slot-0 rows of x, so no collectives are needed.
Each core returns its [4, 1032, 1024] slice of y.

Host-side prep (pure layout/dtype transforms, no model math): the x
slice is cast to bf16 and pre-transposed to feature-on-partition layout
[128, D/128, T]; weights are cast to bf16 and pre-arranged into the
SBUF layouts the matmuls want (Wq/Wk head-interleaved); everything is
packed into ONE flat bf16 input buffer per core (fewer buffer handles
per dispatch = less per-call marshalling).

On-device pipeline (all matmuls bf16 inputs, fp32 PSUM accumulation):
  - qT/kT = W^T @ xT stay feature-on-partition; v = x@Wv
    token-on-partition.
  - scores are computed transposed, sT[j, i] = k_j . q_i, so the
    attention-weights matmul (PV) needs no transposes; softmax
    denominators come from a ones-vector matmul; exp runs on the scalar
    engine reading PSUM directly (scale=1/sqrt(DK) folded in). Scores
    here are O(1) so the max-subtraction is skipped (exp safe in fp32).
  - normalization multiplies the PV output by a broadcast reciprocal
    (broadcast across partitions via a tiny selector matmul).
  - y = outT^T @ Wo + bo, bias added during the PSUM->SBUF copy;
    y is stored bf16 and upcast to fp32 on the host.
"""

import numpy as np

H, BLK, DK, DV = 8, 129, 64, 64
B, N, D = 4, 8256, 1024
INNER = H * DK            # 512
NB = N // BLK             # 64 blocks
NCORES = 8
NBC = NB // NCORES        # 8 blocks per core
T = NBC * BLK             # 1032 tokens per core per batch
DC = D // 128             # 8 contraction chunks over D
FC = INNER // 128         # 4 chunks over the 512 inner dim

# pack buffer element offsets (bf16 elements)
SZ_XCT = B * 128 * DC * T
SZ_XGT = B * 128 * DC * NB
SZ_W = 128 * DC * INNER   # == 128 * FC * D for wo
OFF_XCT = 0
OFF_XGT = OFF_XCT + SZ_XCT
OFF_WQ = OFF_XGT + SZ_XGT
OFF_WK = OFF_WQ + SZ_W
OFF_WV = OFF_WK + SZ_W
OFF_WO = OFF_WV + SZ_W
OFF_BO = OFF_WO + SZ_W
PACK_TOTAL = OFF_BO + D

_NC_CACHE = {}


def _build_nc(variant=0):
    import concourse.bacc as bacc
    import concourse.tile as tile
    from concourse import mybir
    import concourse.bass as bass

    f32 = mybir.dt.float32
    bf16 = mybir.dt.bfloat16

    nc = bacc.Bacc("TRN2", target_bir_lowering=False, debug=False,
                   num_devices=NCORES)

    pack = nc.dram_tensor("pack", [PACK_TOTAL], bf16,
                          kind="ExternalInput").ap()
    y = nc.dram_tensor("y", [B, T, D], bf16, kind="ExternalOutput").ap()

    def sub(off, dims):
        """AP view into the flat pack: dims = [(stride, size), ...]."""
        return bass.AP(tensor=pack.tensor, offset=off,
                       ap=[[s, n] for s, n in dims])

    # token slices for the projection matmuls (psum free dim <= 512)
    TSL = [(0, 512), (512, 512), (1024, T - 1024)]
    # token chunks for the output projection
    TCH = [(i * 128, 128) for i in range(T // 128)] + [(T - T % 128, T % 128)]

    with tile.TileContext(nc) as tc:
        with (
            tc.tile_pool(name="const", bufs=1) as const,
            tc.tile_pool(name="batch", bufs=2) as bp,
            tc.tile_pool(name="stream", bufs=3) as sp,
            tc.tile_pool(name="att", bufs=3) as ap_,
            tc.tile_pool(name="ppsum", bufs=3, space="PSUM") as pp,
            tc.tile_pool(name="spsum", bufs=2, space="PSUM") as stp,
            tc.tile_pool(name="smpsum", bufs=3, space="PSUM") as smp,
        ):
            # ---- constants ----
            ones_col = const.tile([128, 1], bf16)
            nc.vector.memset(ones_col, 1.0)
            ones_row = const.tile([1, 128], bf16)
            nc.vector.memset(ones_row, 1.0)
            # e2_32: partition-broadcast selector for head pairs. Row 32*hp
            # selects cols 0:64 (head 2hp), row 32*hp+1 selects cols 64:128
            # (head 2hp+1); rows are 32-aligned to satisfy PE tile_position.
            import ml_dtypes
            e2_np = np.zeros((128, 128), dtype=ml_dtypes.bfloat16)
            for hp in range(3):
                e2_np[32 * hp, 0:64] = 1.0
                e2_np[32 * hp + 1, 64:128] = 1.0
            e2_dram = nc.inline_tensor(e2_np, name="e2const")
            e2 = const.tile([128, 128], bf16)
            nc.sync.dma_start(out=e2, in_=e2_dram.ap())
            # persistent reciprocal-broadcast staging tile: pairs of rl rows
            # live at partitions {32hp, 32hp+1}; the rest stays zero.
            rl32 = const.tile([128, BLK], bf16)
            nc.vector.memset(rl32, 0.0)

            wq_sb = const.tile([128, DC, INNER], bf16)
            wk_sb = const.tile([128, DC, INNER], bf16)
            wv_sb = const.tile([128, DC, INNER], bf16)
            wo_sb = const.tile([128, FC, D], bf16)
            for w_sb, off in ((wq_sb, OFF_WQ), (wk_sb, OFF_WK)):
                nc.sync.dma_start(
                    out=w_sb,
                    in_=sub(off, [(DC * INNER, 128), (INNER, DC),
                                  (1, INNER)]))
            nc.scalar.dma_start(
                out=wv_sb,
                in_=sub(OFF_WV, [(DC * INNER, 128), (INNER, DC), (1, INNER)]))
            nc.scalar.dma_start(
                out=wo_sb,
                in_=sub(OFF_WO, [(FC * D, 128), (D, FC), (1, D)]))
            bo_bc = const.tile([128, D], bf16)
            nc.sync.dma_start(out=bo_bc, in_=sub(OFF_BO, [(0, 128), (1, D)]))

            for b in range(B):
                # ---- load pre-transposed x / global tokens ----
                xT = bp.tile([128, DC, T], bf16, tag="xT")
                nc.scalar.dma_start(
                    out=xT,
                    in_=sub(OFF_XCT + b * 128 * DC * T,
                            [(DC * T, 128), (T, DC), (1, T)]))
                xgT = bp.tile([128, DC, NB], bf16, tag="xgT")
                nc.sync.dma_start(
                    out=xgT,
                    in_=sub(OFF_XGT + b * 128 * DC * NB,
                            [(DC * NB, 128), (NB, DC), (1, NB)]))

                # ---- global tokens: kgT, vg ----
                kgT = bp.tile([128, FC, NB], bf16, tag="kgT")
                for mc in range(FC):
                    pt = pp.tile([128, 512], f32, tag="pp")
                    for dc in range(DC):
                        nc.tensor.matmul(
                            pt[:, :NB],
                            wk_sb[:, dc, mc * 128:(mc + 1) * 128],
                            xgT[:, dc, :],
                            start=(dc == 0), stop=(dc == DC - 1))
                    nc.vector.tensor_copy(out=kgT[:, mc, :], in_=pt[:, :NB])
                vg = bp.tile([64, INNER], bf16, tag="vg")
                pt = pp.tile([128, 512], f32, tag="pp")
                for dc in range(DC):
                    nc.tensor.matmul(pt[:64, :], xgT[:, dc, 0:64],
                                     wv_sb[:, dc, :],
                                     start=(dc == 0), stop=(dc == DC - 1))
                nc.vector.tensor_copy(out=vg, in_=pt[:64, :])

                # ---- q/k projections (transposed layout) ----
                qT = bp.tile([128, FC, T], bf16, tag="qT")
                kT = bp.tile([128, FC, T], bf16, tag="kT")
                for dst, w_sb, eng in ((qT, wq_sb, "act"), (kT, wk_sb, "act")):
                    for mc in range(FC):
                        for t0, tsz in TSL:
                            pt = pp.tile([128, 512], f32, tag="pp")
                            for dc in range(DC):
                                nc.tensor.matmul(
                                    pt[:, :tsz],
                                    w_sb[:, dc, mc * 128:(mc + 1) * 128],
                                    xT[:, dc, t0:t0 + tsz],
                                    start=(dc == 0), stop=(dc == DC - 1))
                            if eng == "act":
                                nc.scalar.copy(
                                    out=dst[:, mc, t0:t0 + tsz],
                                    in_=pt[:, :tsz])
                            else:
                                nc.vector.tensor_copy(
                                    out=dst[:, mc, t0:t0 + tsz],
                                    in_=pt[:, :tsz])

                # ---- v projection (token-on-partition, per block) ----
                v = bp.tile([128, NBC, INNER], bf16, tag="v")
                for n in range(NBC):
                    pt = pp.tile([128, 512], f32, tag="pp")
                    for dc in range(DC):
                        nc.tensor.matmul(
                            pt, xT[:, dc, n * BLK:n * BLK + 128],
                            wv_sb[:, dc, :],
                            start=(dc == 0), stop=(dc == DC - 1))
                    nc.vector.tensor_copy(out=v[:, n, :], in_=pt)
                # last token of each block, batched: tokens 129n+128
                vl8 = bp.tile([NBC, INNER], bf16, tag="vl8")
                pt = pp.tile([128, 512], f32, tag="pp")
                for dc in range(DC):
                    nc.tensor.matmul(pt[:NBC, :], xT[:, dc, 128::BLK],
                                     wv_sb[:, dc, :],
                                     start=(dc == 0), stop=(dc == DC - 1))
                nc.vector.tensor_copy(out=vl8, in_=pt[:NBC, :])
                vl_all = bp.tile([1, NBC, INNER], bf16, tag="vlall")
                nc.gpsimd.dma_start(out=vl_all, in_=vl8)

                outT = bp.tile([128, FC, T], bf16, tag="outT")

                # ---- global attention for this core's 8 blocks ----
                eg = bp.tile([64, H, NBC], bf16, tag="eg")
                lg = smp.tile([1, H * NBC], f32, tag="sm")
                for h in range(H):
                    p0 = 64 * (h // 4)
                    hc = h % 4
                    sg = smp.tile([64, NBC], f32, tag="sm")
                    nc.tensor.matmul(sg, kgT[p0:p0 + 64, hc, :],
                                     qT[p0:p0 + 64, hc, 0::BLK],
                                     start=True, stop=True)
                    nc.scalar.activation(
                        out=eg[:, h, :], in_=sg,
                        func=mybir.ActivationFunctionType.Exp, scale=0.125)
                    nc.tensor.matmul(lg[:, h * NBC:(h + 1) * NBC],
                                     ones_col[0:64, :], eg[:, h, :],
                                     start=True, stop=True)
                rlg = bp.tile([1, H * NBC], bf16, tag="rlg")
                with nc.allow_low_precision("1/l to bf16"):
                    nc.vector.reciprocal(out=rlg, in_=lg)
                ogn = bp.tile([128, FC, NBC], bf16, tag="ogn")
                for hp in range(4):
                    ogg = smp.tile([128, NBC], f32, tag="sm")
                    for hh in range(2):
                        h = 2 * hp + hh
                        nc.tensor.matmul(
                            ogg[64 * hh:64 * hh + 64, :],
                            vg[:, h * DV:(h + 1) * DV], eg[:, h, :],
                            start=True, stop=True)
                    rlbg = smp.tile([128, NBC], f32, tag="sm")
                    for hh in range(2):
                        o0 = hp * 2 * NBC + hh * NBC
                        nc.tensor.matmul(
                            rlbg[64 * hh:64 * hh + 64, :],
                            ones_row[0:1, 0:64],
                            rlg[0:1, o0:o0 + NBC],
                            start=True, stop=True)
                    rlbg_sb = bp.tile([128, NBC], bf16, tag="rlbg_sb")
                    nc.scalar.copy(out=rlbg_sb, in_=rlbg)
                    nc.vector.tensor_mul(out=ogn[:, hp, :], in0=ogg,
                                         in1=rlbg_sb)

                # ---- block-local attention ----
                for n in range(NBC):
                    c0 = n * BLK
                    eT = ap_.tile([128, H, BLK], bf16, tag="eT")
                    eTl = ap_.tile([1, H, BLK], bf16, tag="eTl")
                    rl = ap_.tile([1, H * BLK], bf16, tag="rl")
                    for hp in range(4):
                        st = stp.tile([128, 2 * BLK], f32, tag="st")
                        stl = smp.tile([1, 2 * BLK], f32, tag="sm")
                        for hh in range(2):
                            h = 2 * hp + hh
                            p0 = 64 * (h // 4)
                            hc = h % 4
                            lq = qT[p0:p0 + 64, hc, c0:c0 + BLK]
                            nc.tensor.matmul(
                                st[:, hh * BLK:(hh + 1) * BLK],
                                kT[p0:p0 + 64, hc, c0:c0 + 128], lq,
                                start=True, stop=True)
                            nc.tensor.matmul(
                                stl[:, hh * BLK:(hh + 1) * BLK],
                                kT[p0:p0 + 64, hc, c0 + 128:c0 + BLK], lq,
                                start=True, stop=True)
                        ex = mybir.ActivationFunctionType.Exp
                        nc.scalar.activation(
                            out=eT[:, 2 * hp:2 * hp + 2, :], in_=st,
                            func=ex, scale=0.125)
                        nc.scalar.activation(
                            out=eTl[:, 2 * hp:2 * hp + 2, :], in_=stl,
                            func=ex, scale=0.125)
                        lp = smp.tile([1, 2 * BLK], f32, tag="sm")
                        nc.tensor.matmul(lp, ones_col,
                                         eT[:, 2 * hp:2 * hp + 2, :],
                                         start=True, stop=False)
                        nc.tensor.matmul(lp, ones_col[0:1, :],
                                         eTl[:, 2 * hp:2 * hp + 2, :],
                                         start=False, stop=True)
                        with nc.allow_low_precision(
                                "1/l to bf16, matches prior cast-DMA"):
                            nc.vector.reciprocal(
                                out=rl[:, hp * 2 * BLK:(hp + 1) * 2 * BLK],
                                in_=lp)
                    # reshape rl to head-pair-on-partition rows 32hp, 32hp+1
                    # (hp 0..2 only; base partition 96 is not allowed)
                    rlv = rl.rearrange("o (a m i) -> o a m i", a=4, m=2)
                    for hp in range(3):
                        nc.gpsimd.dma_start(
                            out=rl32[32 * hp:32 * hp + 2, :],
                            in_=rlv[:, hp, :, :])
                    for hp in range(4):
                        og = smp.tile([128, BLK], f32, tag="sm")
                        for hh in range(2):
                            h = 2 * hp + hh
                            nc.tensor.matmul(
                                og[64 * hh:64 * hh + 64, :],
                                v[:, n, h * DV:(h + 1) * DV],
                                eT[:, h, :], start=True, stop=False)
                            nc.tensor.matmul(
                                og[64 * hh:64 * hh + 64, :],
                                vl_all[0:1, n, h * DV:(h + 1) * DV],
                                eTl[:, h, :], start=False, stop=True)
                        rlb = smp.tile([128, BLK], f32, tag="sm")
                        if hp < 3:
                            nc.tensor.matmul(
                                rlb, e2[32 * hp:32 * hp + 2, :],
                                rl32[32 * hp:32 * hp + 2, :],
                                start=True, stop=True)
                        else:
                            for hh in range(2):
                                o0 = hp * 2 * BLK + hh * BLK
                                nc.tensor.matmul(
                                    rlb[64 * hh:64 * hh + 64, :],
                                    ones_row[0:1, 0:64],
                                    rl[0:1, o0:o0 + BLK],
                                    start=True, stop=True)
                        rlb_sb = ap_.tile([128, BLK], bf16, tag="rlb_sb")
                        nc.scalar.copy(out=rlb_sb, in_=rlb)
                        nc.vector.tensor_mul(
                            out=outT[:, hp, c0:c0 + BLK], in0=og,
                            in1=rlb_sb)
                        nc.gpsimd.tensor_add(
                            out=outT[:, hp, c0:c0 + 1],
                            in0=outT[:, hp, c0:c0 + 1],
                            in1=ogn[:, hp, n:n + 1])

                # ---- output projection + bias ----
                for t0, tsz in TCH:
                    ysb = sp.tile([128, D], bf16, tag="ysb")
                    for half in range(2):
                        f0 = half * 512
                        pt = pp.tile([128, 512], f32, tag="pp")
                        for fc in range(FC):
                            nc.tensor.matmul(
                                pt[:tsz, :],
                                outT[:, fc, t0:t0 + tsz],
                                wo_sb[:, fc, f0:f0 + 512],
                                start=(fc == 0), stop=(fc == FC - 1))
                        with nc.allow_low_precision("y stored bf16"):
                            nc.vector.tensor_add(
                                out=ysb[:tsz, f0:f0 + 512], in0=pt[:tsz, :],
                                in1=bo_bc[:tsz, f0:f0 + 512])
                    nc.sync.dma_start(out=y[b, t0:t0 + tsz, :],
                                      in_=ysb[:tsz, :])

    nc.compile()
    return nc


def _get_nc():
    if "nc" not in _NC_CACHE:
        _NC_CACHE["nc"] = _build_nc()
    return _NC_CACHE["nc"]


def prepare_in_maps(x, Wq, Wk, Wv, Wo, bo):
    """Host-side layout/dtype prep: returns per-core {'pack': flat bf16}."""
    import ml_dtypes

    bf = ml_dtypes.bfloat16
    x = np.asarray(x, dtype=np.float32)
    Wq, Wk, Wv, Wo = (np.asarray(w, np.float32) for w in (Wq, Wk, Wv, Wo))
    bo = np.asarray(bo, np.float32).reshape(D)

    def qk_interleave(W):
        # w_sb[p, c, m*128 + 64*a + d] = W[c*128 + p, a*256 + m*64 + d]
        Wv5 = W.reshape(DC, 128, 2, 4, 64)
        return np.ascontiguousarray(
            Wv5.transpose(1, 0, 3, 2, 4)).reshape(128, DC * INNER).astype(bf)

    wq_i = qk_interleave(Wq)
    wk_i = qk_interleave(Wk)
    wv_s = np.ascontiguousarray(
        Wv.reshape(DC, 128, INNER).transpose(1, 0, 2)).reshape(
            128, DC * INNER).astype(bf)
    wo_s = np.ascontiguousarray(
        Wo.reshape(FC, 128, D).transpose(1, 0, 2)).reshape(
            128, FC * D).astype(bf)
    bo_b = bo.astype(bf)

    xg = x[:, ::BLK, :]                                   # [B, NB, D]
    xgt = np.ascontiguousarray(
        xg.reshape(B, NB, DC, 128).transpose(0, 3, 2, 1)).astype(bf)

    const_part = np.concatenate(
        [a.reshape(-1) for a in (wq_i, wk_i, wv_s, wo_s, bo_b)])

    in_maps = []
    for c in range(NCORES):
        xc = x[:, c * T:(c + 1) * T, :]                   # [B, T, D]
        xct = np.ascontiguousarray(
            xc.reshape(B, T, DC, 128).transpose(0, 3, 2, 1)).astype(bf)
        packv = np.concatenate(
            [xct.reshape(-1), xgt.reshape(-1), const_part])
        assert packv.shape[0] == PACK_TOTAL
        in_maps.append({"pack": packv})
    return in_maps


def kernel(x, Wq, Wk, Wv, Wo, bo):
    from concourse.bass_utils import run_bass_kernel_spmd

    nc = _get_nc()
    in_maps = prepare_in_maps(x, Wq, Wk, Wv, Wo, bo)
    res = run_bass_kernel_spmd(nc, in_maps, core_ids=list(range(NCORES)))
    return np.concatenate(
        [res.results[c]["y"] for c in range(NCORES)],
        axis=1).astype(np.float32)
